# revision 16
# baseline (speedup 1.0000x reference)
"""Trainium2 Bass kernel for the conv1d-stack + MLP ragged-sequence model.

Strategy:
  - Pure data parallelism over 8 NeuronCores: 8 samples per core.
  - Samples are sorted by valid length (descending) and dealt round-robin to
    cores, so every core runs an IDENTICAL (SPMD) program whose per-slot
    sizes are the per-rank-group maximum length.  This exploits raggedness:
    work scales with sum of group maxima (~0.67x of padded full length).
  - Convs are TensorEngine matmuls in float32r (4x fp32 rate at N>=256) with
    per-tap PSUM accumulation.  Conv1 consumes a host-prepared shift-stacked
    [120, T] input (3-tap packing -> 4 matmul groups) or a polyphase [80,T/2]
    layout (5 groups).  Layers 5-7 batch all 8 slots into single wide
    matmuls.  Bias+ReLU eviction on ScalarE, avgpool on VectorE.
  - Slots are emitted in pairs (j, j+1 interleaved layer-by-layer) so PSUM
    eviction latency of slot j hides behind slot j+1's matmuls.
  - The ragged masked-max is applied with a host-built additive -1e30 mask
    (a data input, so it is SPMD-safe), then the 3-layer MLP runs on-device.
"""

import os
import sys

for _p in ("/opt/trn_rl_repo",):
    if _p not in sys.path and os.path.isdir(_p):
        sys.path.insert(0, _p)

import numpy as np

import concourse.bass as bass  # noqa: F401  (registers types)
from concourse import bacc
import concourse.tile as tile
import concourse.mybir as mybir
from concourse.bass_utils import run_bass_kernel_spmd

F32 = mybir.dt.float32
F32R = mybir.dt.float32r
AF = mybir.ActivationFunctionType
AX = mybir.AxisListType

N_CORES = 8
N_SLOTS = 8
B = 64
C_IN = 40
T_FULL = 8192
NEG = -1.0e30
L1_STACK = True  # True: [120,T] 3-shift stack, 4 L1 groups; False: polyphase 80, 5 groups


def _chain(t0):
    """Per-layer valid/capacity length chain (mirrors the reference)."""
    t1 = (t0 - 10) // 2 + 1
    t2 = (t1 - 5) // 2 + 1
    t3 = (t2 - 5) // 2 + 1
    t4 = (t3 - 5) // 2 + 1
    t4p = t4 // 2
    t5 = (t4p - 5) // 2 + 1
    t5p = t5 // 2
    t6 = (t5p - 5) // 2 + 1
    t7 = (t6 - 3) // 2 + 1
    return t1, t2, t3, t4, t4p, t5, t5p, t6, t7


def _uniform_tail(caps):
    T4P0 = _chain(caps[0])[4]
    T5u = (T4P0 - 5) // 2 + 1
    T5pu = T5u // 2
    T6u = (T5pu - 5) // 2 + 1
    T7u = (T6u - 3) // 2 + 1
    return T4P0, T5u, T5pu, T6u, T7u


def _build_program(caps):
    """Emit the SPMD Bass/Tile program for the given 8 slot capacities."""
    T0 = list(caps)
    T1, T2, T3, T4, T4p = [], [], [], [], []
    for t0 in T0:
        t1, t2, t3, t4, t4p, _, _, _, _ = _chain(t0)
        T1.append(t1)
        T2.append(t2)
        T3.append(t3)
        T4.append(t4)
        T4p.append(t4p)
    T4P0, T5u, T5pu, T6u, T7u = _uniform_tail(caps)

    nc = bacc.Bacc("TRN2", target_bir_lowering=False, debug=False)

    x_rows = 120 if L1_STACK else 80
    xs = [
        nc.dram_tensor(f"x{j}", [x_rows, (T0[j] if L1_STACK else T0[j] // 2)], F32R,
                       kind="ExternalInput")
        for j in range(N_SLOTS)
    ]
    w1_t = nc.dram_tensor("w1s", [x_rows, (4 if L1_STACK else 5) * 96], F32R,
                          kind="ExternalInput")
    wl_t = {
        l: nc.dram_tensor(f"w{l}s", [96, 5 * 96], F32R, kind="ExternalInput")
        for l in (2, 3, 4, 5, 6)
    }
    w7_t = nc.dram_tensor("w7s", [96, 3 * 128], F32R, kind="ExternalInput")
    lw1_t = nc.dram_tensor("lw1T", [128, 128], F32R, kind="ExternalInput")
    lw2_t = nc.dram_tensor("lw2T", [128, 64], F32R, kind="ExternalInput")
    lw3_t = nc.dram_tensor("lw3T", [64, 5], F32R, kind="ExternalInput")
    bias_t = nc.dram_tensor("biases", [128, 10], F32, kind="ExternalInput")
    fmask_t = nc.dram_tensor("fmask", [128, N_SLOTS * T7u], F32, kind="ExternalInput")
    out_t = nc.dram_tensor("out", [5, N_SLOTS], F32, kind="ExternalOutput")

    with tile.TileContext(nc) as tc:
        with (
            tc.tile_pool(name="wp", bufs=1) as wp,
            tc.tile_pool(name="xp", bufs=3) as xp,
            tc.tile_pool(name="actp", bufs=2) as ap_,
            tc.tile_pool(name="catp", bufs=1) as cp,
            tc.tile_pool(name="psp", bufs=2, space="PSUM") as pp,
        ):
            # ---- resident weights / constants ----
            w1s = wp.tile([x_rows, (4 if L1_STACK else 5) * 96], F32R, tag="w1")
            nc.sync.dma_start(w1s[:], w1_t[:])
            wls = {}
            for l in (2, 3, 4, 5, 6):
                wls[l] = wp.tile([96, 5 * 96], F32R, tag=f"w{l}", name=f"w{l}sb")
                nc.sync.dma_start(wls[l][:], wl_t[l][:])
            w7s = wp.tile([96, 3 * 128], F32R, tag="w7")
            nc.sync.dma_start(w7s[:], w7_t[:])
            lw1s = wp.tile([128, 128], F32R, tag="lw1")
            nc.sync.dma_start(lw1s[:], lw1_t[:])
            lw2s = wp.tile([128, 64], F32R, tag="lw2")
            nc.sync.dma_start(lw2s[:], lw2_t[:])
            lw3s = wp.tile([64, 5], F32R, tag="lw3")
            nc.sync.dma_start(lw3s[:], lw3_t[:])
            bs = wp.tile([128, 10], F32, tag="bias")
            nc.sync.dma_start(bs[:], bias_t[:])
            fms = wp.tile([128, N_SLOTS * T7u], F32, tag="fmask")
            nc.sync.dma_start(fms[:], fmask_t[:])

            # ---- concatenated small-layer buffers ----
            p4cat = cp.tile([96, N_SLOTS * T4P0], F32R, tag="p4cat")
            nc.gpsimd.memset(p4cat[:].bitcast(F32), 0.0)
            p5cat = cp.tile([96, N_SLOTS * T5pu], F32R, tag="p5cat")
            b6cat = cp.tile([96, N_SLOTS * T6u], F32R, tag="b6cat")
            b7cat = cp.tile([128, N_SLOTS * T7u], F32, tag="b7cat")

            def conv_evict(dst_ap, psum_ap, bias_col, func=AF.Relu, prange=96):
                nc.scalar.activation(
                    dst_ap, psum_ap, func, bias=bs[0:prange, bias_col : bias_col + 1]
                )

            # per-slot state for paired emission
            x2t = [None] * N_SLOTS
            buf1t = [None] * N_SLOTS
            buf2t = [None] * N_SLOTS
            buf3t = [None] * N_SLOTS

            def emit_load(j):
                if L1_STACK:
                    x2 = xp.tile([120, T0[j]], F32R, tag="x2", name=f"x2_{j}")
                    # split load so DMA parallelism spans more engines/queues
                    nc.sync.dma_start(x2[0:40, :], xs[j][0:40, :])
                    nc.sync.dma_start(x2[40:80, :], xs[j][40:80, :])
                    nc.sync.dma_start(x2[80:120, :], xs[j][80:120, :])
                else:
                    th = T0[j] // 2
                    x2 = xp.tile([80, th], F32R, tag="x2", name=f"x2_{j}")
                    nc.sync.dma_start(x2[0:40, :], xs[j][0:40, :])
                    nc.sync.dma_start(x2[40:80, :], xs[j][40:80, :])
                x2t[j] = x2

            def emit_l1(j):
                x2 = x2t[j]
                t1 = T1[j]
                buf1 = ap_.tile([96, t1], F32R, tag="b1", name=f"b1_{j}")
                for ts_ in range(0, t1, 2048):
                    cols = min(2048, t1 - ts_)
                    ps = pp.tile([96, 2048], F32, tag="conv", name=f"ps1_{j}_{ts_}")
                    if L1_STACK:
                        glist = ((0, 120), (1, 120), (2, 120), (3, 40))
                    else:
                        glist = ((0, 80), (1, 80), (2, 80), (3, 80), (4, 80))
                    glast = len(glist) - 1
                    for g, kk in glist:
                        lhsT = w1s[0:kk, 96 * g : 96 * (g + 1)]
                        for cs in range(0, cols, 512):
                            cn = min(512, cols - cs)
                            t_off = ts_ + cs
                            if L1_STACK:
                                rhs = x2[0:kk, 3 * g + 2 * t_off : 3 * g + 2 * (t_off + cn) - 1 : 2]
                            else:
                                rhs = x2[0:kk, t_off + g : t_off + g + cn]
                            nc.tensor.matmul(
                                ps[0:96, cs : cs + cn], lhsT, rhs,
                                start=(g == 0), stop=(g == glast),
                            )
                    conv_evict(buf1[:, ts_ : ts_ + cols], ps[0:96, 0:cols], 0)
                buf1t[j] = buf1

            def emit_conv(j, src_list, dst_list, w_sb, tin_all, tout_all, bias_col):
                tout = tout_all[j]
                src = src_list[j]
                dst = ap_.tile([96, tout], F32R, tag=f"bo{bias_col}", name=f"bo{bias_col}_{j}")
                for ts_ in range(0, tout, 2048):
                    cols = min(2048, tout - ts_)
                    ps = pp.tile([96, 2048], F32, tag="conv", name=f"psc{bias_col}_{j}_{ts_}")
                    for g in range(5):
                        lhsT = w_sb[:, 96 * g : 96 * (g + 1)]
                        for cs in range(0, cols, 512):
                            cn = min(512, cols - cs)
                            t0 = ts_ + cs
                            nc.tensor.matmul(
                                ps[0:96, cs : cs + cn],
                                lhsT,
                                src[0:96, g + 2 * t0 : g + 2 * (t0 + cn) - 1 : 2],
                                start=(g == 0),
                                stop=(g == 4),
                            )
                    conv_evict(dst[:, ts_ : ts_ + cols], ps[0:96, 0:cols], bias_col)
                dst_list[j] = dst

            def emit_l4pool(j):
                t4 = T4[j]
                t4p = T4p[j]
                ps = pp.tile([96, 512], F32, tag="conv", name=f"ps4_{j}")
                for g in range(5):
                    nc.tensor.matmul(
                        ps[0:96, 0:t4],
                        wls[4][:, 96 * g : 96 * (g + 1)],
                        buf3t[j][0:96, g : g + 2 * t4 - 1 : 2],
                        start=(g == 0),
                        stop=(g == 4),
                    )
                buf4 = ap_.tile([96, t4], F32, tag="b4", name=f"b4_{j}")
                nc.scalar.activation(buf4[:, 0:t4], ps[0:96, 0:t4], AF.Copy)
                tmp4 = ap_.tile([96, t4p], F32, tag="t4", name=f"t4_{j}")
                nc.vector.tensor_add(
                    tmp4[:, 0:t4p],
                    buf4[0:96, 0 : 2 * t4p : 2],
                    buf4[0:96, 1 : 2 * t4p : 2],
                )
                # relu(even+odd+2*b4); the 0.5 of the avg-pool is folded into w5
                conv_evict(p4cat[:, j * T4P0 : j * T4P0 + t4p], tmp4[0:96, 0:t4p], 3)

            # ---- paired slot emission: evictions hide behind partner MMs ----
            for p in range(0, N_SLOTS, 2):
                a, b = p, p + 1
                if p == 0:
                    emit_load(a)
                    emit_load(b)
                emit_l1(a)
                emit_l1(b)
                if p + 2 < N_SLOTS:
                    emit_load(p + 2)
                    emit_load(p + 3)
                emit_conv(a, buf1t, buf2t, wls[2], T1, T2, 1)
                emit_conv(b, buf1t, buf2t, wls[2], T1, T2, 1)
                emit_conv(a, buf2t, buf3t, wls[3], T2, T3, 2)
                emit_conv(b, buf2t, buf3t, wls[3], T2, T3, 2)
                emit_l4pool(a)
                emit_l4pool(b)

            # ---- batched tail layers over all 8 slots ----
            # Layouts from here on are slot-interleaved: column = t*8 + s, so
            # the innermost matmul AP dim is the slot dim (count 8, even —
            # an fp32r ISA requirement).
            p4tv = p4cat[:].rearrange("p (s t) -> p t s", s=N_SLOTS)  # [p, T4P0, 8]
            ps5 = pp.tile([96, 1024], F32, tag="conv")
            for g in range(5):
                lhsT = wls[5][:, 96 * g : 96 * (g + 1)]
                for ci, tb in enumerate(range(0, T5u, 64)):
                    tn = min(64, T5u - tb)
                    nc.tensor.matmul(
                        ps5[0:96, 512 * ci : 512 * ci + 8 * tn],
                        lhsT,
                        p4tv[0:96, g + 2 * tb : g + 2 * (tb + tn) - 1 : 2, :],
                        start=(g == 0),
                        stop=(g == 4),
                    )
            # buf5 col = t*8+s, contiguous across the two psum chunks
            buf5 = ap_.tile([96, N_SLOTS * T5u], F32, tag="b5")
            for ci, tb in enumerate(range(0, T5u, 64)):
                tn = min(64, T5u - tb)
                nc.scalar.activation(
                    buf5[:, 8 * tb : 8 * (tb + tn)],
                    ps5[0:96, 512 * ci : 512 * ci + 8 * tn],
                    AF.Copy,
                )
            b5v = buf5[:].rearrange("p (t s) -> p t s", s=N_SLOTS)
            tmp5 = ap_.tile([96, N_SLOTS * T5pu], F32, tag="t5")
            nc.vector.tensor_add(
                tmp5[:].rearrange("p (t s) -> p t s", s=N_SLOTS),
                b5v[:, 0 : 2 * T5pu : 2, :],
                b5v[:, 1 : 2 * T5pu : 2, :],
            )
            conv_evict(p5cat[:], tmp5[0:96, :], 4)  # p5cat col = u*8+s

            p5tv = p5cat[:].rearrange("p (t s) -> p t s", s=N_SLOTS)
            ps6 = pp.tile([96, N_SLOTS * T6u], F32, tag="conv")
            for g in range(5):
                nc.tensor.matmul(
                    ps6[0:96, :],
                    wls[6][:, 96 * g : 96 * (g + 1)],
                    p5tv[0:96, g : g + 2 * T6u - 1 : 2, :],
                    start=(g == 0),
                    stop=(g == 4),
                )
            conv_evict(b6cat[:], ps6[0:96, :], 5)  # b6cat col = t*8+s

            b6tv = b6cat[:].rearrange("p (t s) -> p t s", s=N_SLOTS)
            ps7 = pp.tile([128, N_SLOTS * T7u], F32, tag="conv")
            for g in range(3):
                nc.tensor.matmul(
                    ps7[0:128, :],
                    w7s[:, 128 * g : 128 * (g + 1)],
                    b6tv[0:96, g : g + 2 * T7u - 1 : 2, :],
                    start=(g == 0),
                    stop=(g == 2),
                )
            conv_evict(b7cat[:], ps7[0:128, :], 6, prange=128)  # col = t*8+s

            # ---- ragged masked max + MLP head ----
            tmpm = ap_.tile([128, N_SLOTS * T7u], F32, tag="tm")
            nc.vector.tensor_add(tmpm[:], b7cat[:], fms[:])
            xmax = ap_.tile([128, N_SLOTS], F32R, tag="xmax")
            nc.vector.reduce_max(
                xmax[:],
                tmpm[:].rearrange("p (t s) -> p s t", s=N_SLOTS),
                axis=AX.X,
            )

            psm1 = pp.tile([128, N_SLOTS], F32, tag="conv")
            nc.tensor.matmul(psm1[0:128, :], lw1s[:], xmax[:], start=True, stop=True)
            h1 = ap_.tile([128, N_SLOTS], F32R, tag="h1")
            conv_evict(h1[:], psm1[0:128, :], 7, prange=128)

            psm2 = pp.tile([64, N_SLOTS], F32, tag="conv")
            nc.tensor.matmul(psm2[0:64, :], lw2s[:], h1[:], start=True, stop=True)
            h2 = ap_.tile([64, N_SLOTS], F32R, tag="h2")
            conv_evict(h2[:], psm2[0:64, :], 8, prange=64)

            psm3 = pp.tile([5, N_SLOTS], F32, tag="conv")
            nc.tensor.matmul(psm3[0:5, :], lw3s[:], h2[0:64, :], start=True, stop=True)
            outsb = ap_.tile([5, N_SLOTS], F32, tag="osb")
            nc.vector.tensor_scalar_add(outsb[:], psm3[0:5, :], bs[0:5, 9:10])
            nc.sync.dma_start(out_t[:], outsb[:])

    nc.compile()
    return nc


def _prep_x(x, b, cap):
    """Host-side input re-layout for one sample/slot."""
    xb = np.asarray(x[b, :, :cap], np.float32)
    if L1_STACK:
        s = np.zeros((120, cap), np.float32)
        s[0:40] = xb
        s[40:80, : cap - 1] = xb[:, 1:]
        s[80:120, : cap - 2] = xb[:, 2:]
        return s
    th = cap // 2
    return np.concatenate([xb[:, 0 : 2 * th : 2], xb[:, 1 : 2 * th : 2]], axis=0)


def _prep_weights(inp):
    """Host-side weight/bias re-layout (all tiny)."""
    w = {}
    w1 = np.asarray(inp["w1"], np.float32)  # [96, 40, 10]
    if L1_STACK:
        # stacked: rows (d*40+c), group g: value W1[o, c, 3g+d]  (k = 3g+d)
        w1s = np.zeros((120, 4 * 96), np.float32)
        for g in range(4):
            for d in range(3):
                k = 3 * g + d
                if k > 9:
                    break
                w1s[d * 40 : (d + 1) * 40, 96 * g : 96 * (g + 1)] = w1[:, :, k].T
        w["w1s"] = w1s
    else:
        # polyphase phase-major: rows (p*40+c), cols (m*96+o): W1[o, c, 2m+p]
        w["w1s"] = np.ascontiguousarray(
            w1.transpose(1, 2, 0).reshape(40, 5, 2, 96).transpose(2, 0, 1, 3).reshape(80, 480)
        )
    for l, scale in ((2, 1.0), (3, 1.0), (4, 1.0), (5, 0.5), (6, 0.5)):
        wl = np.asarray(inp[f"w{l}"], np.float32)  # [96, 96, 5]
        w[f"w{l}s"] = np.ascontiguousarray(wl.transpose(1, 2, 0).reshape(96, 480) * scale)
    w7 = np.asarray(inp["w7"], np.float32)  # [128, 96, 3]
    w["w7s"] = np.ascontiguousarray(w7.transpose(1, 2, 0).reshape(96, 384))
    w["lw1T"] = np.ascontiguousarray(np.asarray(inp["lw1"], np.float32).T)  # [128,128]
    w["lw2T"] = np.ascontiguousarray(np.asarray(inp["lw2"], np.float32).T)  # [128,64]
    w["lw3T"] = np.ascontiguousarray(np.asarray(inp["lw3"], np.float32).T)  # [64,5]

    biases = np.zeros((128, 10), np.float32)
    biases[0:96, 0] = np.asarray(inp["b1"], np.float32)
    biases[0:96, 1] = np.asarray(inp["b2"], np.float32)
    biases[0:96, 2] = np.asarray(inp["b3"], np.float32)
    biases[0:96, 3] = 2.0 * np.asarray(inp["b4"], np.float32)
    biases[0:96, 4] = 2.0 * np.asarray(inp["b5"], np.float32)
    biases[0:96, 5] = np.asarray(inp["b6"], np.float32)
    biases[0:128, 6] = np.asarray(inp["b7"], np.float32)
    biases[0:128, 7] = np.asarray(inp["lb1"], np.float32)
    biases[0:64, 8] = np.asarray(inp["lb2"], np.float32)
    biases[0:5, 9] = np.asarray(inp["lb3"], np.float32)
    w["biases"] = biases
    return w


def _schedule(len_mask):
    """Sort samples by length desc, deal round-robin: core c, slot j gets
    sample order[8j + c].  Slot capacity = rank-group max, rounded to even."""
    lens = np.asarray(len_mask, np.int64).clip(1, T_FULL)
    order = np.argsort(-lens, kind="stable")
    sample_of = np.zeros((N_CORES, N_SLOTS), np.int64)
    caps = []
    for j in range(N_SLOTS):
        grp = order[j * N_CORES : (j + 1) * N_CORES]
        for c in range(N_CORES):
            sample_of[c, j] = grp[c]
        cap = int(lens[grp].max())
        cap = max(cap, 1312)  # keep the whole chain >= 1 frame
        # round up to a multiple of 32 so T1..T4 are all even
        # (fp32r matmuls require an even moving-operand size)
        cap = min(((cap + 31) // 32) * 32, T_FULL)
        caps.append(cap)
    return order, sample_of, caps


def _make_in_maps(inputs, sample_of, caps):
    x = np.asarray(inputs["x_input"], np.float32)
    len_mask = np.asarray(inputs["len_mask"], np.int32)
    _, _, _, _, T7u = _uniform_tail(caps)
    w = _prep_weights(inputs)
    in_maps = []
    for c in range(N_CORES):
        m = dict(w)
        # slot-interleaved mask layout: column = t*8 + s
        fm2 = np.full((T7u, N_SLOTS), NEG, np.float32)
        for j in range(N_SLOTS):
            bidx = int(sample_of[c, j])
            m[f"x{j}"] = _prep_x(x, bidx, caps[j])
            lv7 = _chain(int(max(min(len_mask[bidx], T_FULL), 1312)))[8]
            lv7 = max(min(lv7, T7u), 1)
            fm2[0:lv7, j] = 0.0
        fmask = fm2.reshape(-1)
        m["fmask"] = np.ascontiguousarray(
            np.broadcast_to(fmask[None, :], (128, N_SLOTS * T7u))
        )
        in_maps.append(m)
    return in_maps


def _ensure_ntff_hook():
    """The agent image lacks ``antenv.axon_hooks``; seed a shim so
    ``run_bass_kernel_spmd(trace=True)`` can reach the axon NTFF profiler."""
    import types

    if "antenv.axon_hooks" in sys.modules:
        return
    try:
        from trn_agent_boot.trn_boot import _ntff_profile_via_ctypes

        hook = _ntff_profile_via_ctypes("/opt/axon/libaxon_pjrt.so")
    except Exception:
        hook = None
    mod = types.ModuleType("antenv.axon_hooks")
    state = {"hook": hook}
    mod.get_axon_ntff_profile_hook = lambda: state["hook"]
    mod.set_axon_ntff_profile_hook = lambda h: state.update(hook=h)
    sys.modules["antenv.axon_hooks"] = mod


def _run(inputs, trace=False):
    if trace:
        _ensure_ntff_hook()
    len_mask = np.asarray(inputs["len_mask"], np.int32)
    order, sample_of, caps = _schedule(len_mask)
    nc = _build_program(caps)
    in_maps = _make_in_maps(inputs, sample_of, caps)
    res = run_bass_kernel_spmd(
        nc, in_maps, core_ids=list(range(N_CORES)), trace=trace
    )
    out = np.zeros((B, 5), np.float32)
    for c in range(N_CORES):
        o = res.results[c]["out"]  # [5, 8]
        for j in range(N_SLOTS):
            out[int(sample_of[c, j])] = o[:, j]
    return out, res


def kernel(**inputs):
    out, _ = _run(inputs, trace=False)
    return out


# revision 17
# speedup vs baseline: 1.5266x; 1.5266x over previous
"""Trainium2 Bass kernel for the conv1d-stack + MLP ragged-sequence model.

Strategy:
  - Pure data parallelism over 8 NeuronCores: 8 samples per core.
  - Samples are sorted by valid length (descending) and dealt round-robin to
    cores, so every core runs an IDENTICAL (SPMD) program whose per-slot
    sizes are the per-rank-group maximum length (exploits raggedness).
  - Convs run on the TensorEngine as float32r matmuls with per-tap PSUM
    accumulation.  All activations are kept PHASE-SPLIT (even/odd time
    samples in separate buffers), which turns every stride-2 conv into
    stride-1 matmul reads — stride-2 moving operands run the PE at half
    rate, stride-1 reach the full warm rate (~0.45 ns/col measured).
  - Layers 5-7 additionally interleave all 8 slots (column = t*8 + s) so
    their matmul reads are single contiguous slabs.
  - Slots are emitted in pairs so PSUM-eviction latency of slot j hides
    behind slot j+1's matmuls.  Bias+ReLU eviction on ScalarE (which also
    does the phase split via strided reads), avgpool on VectorE.
  - The ragged masked-max uses a host-built additive -1e30 mask (a data
    input, SPMD-safe), then the 3-layer MLP runs on-device.
"""

import os
import sys

for _p in ("/opt/trn_rl_repo",):
    if _p not in sys.path and os.path.isdir(_p):
        sys.path.insert(0, _p)

import numpy as np

import concourse.bass as bass  # noqa: F401  (registers types)
from concourse import bacc
import concourse.tile as tile
import concourse.mybir as mybir
from concourse.bass_utils import run_bass_kernel_spmd

F32 = mybir.dt.float32
F32R = mybir.dt.float32r
AF = mybir.ActivationFunctionType
AX = mybir.AxisListType

N_CORES = 8
N_SLOTS = 8
B = 64
C_IN = 40
T_FULL = 8192
NEG = -1.0e30


def _chain(t0):
    """Per-layer valid/capacity length chain (mirrors the reference)."""
    t1 = (t0 - 10) // 2 + 1
    t2 = (t1 - 5) // 2 + 1
    t3 = (t2 - 5) // 2 + 1
    t4 = (t3 - 5) // 2 + 1
    t4p = t4 // 2
    t5 = (t4p - 5) // 2 + 1
    t5p = t5 // 2
    t6 = (t5p - 5) // 2 + 1
    t7 = (t6 - 3) // 2 + 1
    return t1, t2, t3, t4, t4p, t5, t5p, t6, t7


def _uniform_tail(caps):
    T4P0 = _chain(caps[0])[4]
    T5u = (T4P0 - 5) // 2 + 1
    T5pu = T5u // 2
    T6u = (T5pu - 5) // 2 + 1
    T7u = (T6u - 3) // 2 + 1
    return T4P0, T5u, T5pu, T6u, T7u


def _build_program(caps):
    """Emit the SPMD Bass/Tile program for the given 8 slot capacities."""
    T0 = list(caps)
    T1, T2, T3, T4, T4p = [], [], [], [], []
    for t0 in T0:
        t1, t2, t3, t4, t4p, _, _, _, _ = _chain(t0)
        T1.append(t1)
        T2.append(t2)
        T3.append(t3)
        T4.append(t4)
        T4p.append(t4p)
    T4P0, T5u, T5pu, T6u, T7u = _uniform_tail(caps)
    # phase-split widths of the pooled L4 output (uniform cat geometry)
    P4E = (T4P0 + 1) // 2
    P4O = T4P0 // 2
    P5E = (T5pu + 1) // 2
    P5O = T5pu // 2
    P6E = (T6u + 1) // 2
    P6O = T6u // 2

    nc = bacc.Bacc("TRN2", target_bir_lowering=False, debug=False)

    xs = [
        nc.dram_tensor(f"x{j}", [80, T0[j] // 2], F32R, kind="ExternalInput")
        for j in range(N_SLOTS)
    ]
    w1_t = nc.dram_tensor("w1s", [80, 5 * 96], F32R, kind="ExternalInput")
    wl_t = {
        l: nc.dram_tensor(f"w{l}s", [96, 5 * 96], F32R, kind="ExternalInput")
        for l in (2, 3, 4, 5, 6)
    }
    w7_t = nc.dram_tensor("w7s", [96, 3 * 128], F32R, kind="ExternalInput")
    lw1_t = nc.dram_tensor("lw1T", [128, 128], F32R, kind="ExternalInput")
    lw2_t = nc.dram_tensor("lw2T", [128, 64], F32R, kind="ExternalInput")
    lw3_t = nc.dram_tensor("lw3T", [64, 5], F32R, kind="ExternalInput")
    bias_t = nc.dram_tensor("biases", [128, 10], F32, kind="ExternalInput")
    fmask_t = nc.dram_tensor("fmask", [128, N_SLOTS * T7u], F32, kind="ExternalInput")
    out_t = nc.dram_tensor("out", [5, N_SLOTS], F32, kind="ExternalOutput")

    with tile.TileContext(nc) as tc:
        with (
            tc.tile_pool(name="wp", bufs=1) as wp,
            tc.tile_pool(name="xp", bufs=3) as xp,
            tc.tile_pool(name="actp", bufs=2) as ap_,
            tc.tile_pool(name="catp", bufs=1) as cp,
            tc.tile_pool(name="psp", bufs=2, space="PSUM") as pp,
        ):
            # ---- slot-0/1 inputs first (they gate the first matmuls) ----
            x2t = [None] * N_SLOTS

            def emit_load(j):
                x2 = xp.tile([80, T0[j] // 2], F32R, tag="x2", name=f"x2_{j}")
                nc.sync.dma_start(x2[:], xs[j][:])
                x2t[j] = x2

            emit_load(0)
            emit_load(1)

            # ---- resident weights / constants (scalar HWDGE ring) ----
            w1s = wp.tile([80, 5 * 96], F32R, tag="w1")
            nc.scalar.dma_start(w1s[:], w1_t[:])
            wls = {}
            for l in (2, 3, 4, 5, 6):
                wls[l] = wp.tile([96, 5 * 96], F32R, tag=f"w{l}", name=f"w{l}sb")
                nc.scalar.dma_start(wls[l][:], wl_t[l][:])
            w7s = wp.tile([96, 3 * 128], F32R, tag="w7")
            nc.scalar.dma_start(w7s[:], w7_t[:])
            lw1s = wp.tile([128, 128], F32R, tag="lw1")
            nc.scalar.dma_start(lw1s[:], lw1_t[:])
            lw2s = wp.tile([128, 64], F32R, tag="lw2")
            nc.scalar.dma_start(lw2s[:], lw2_t[:])
            lw3s = wp.tile([64, 5], F32R, tag="lw3")
            nc.scalar.dma_start(lw3s[:], lw3_t[:])
            bs = wp.tile([128, 10], F32, tag="bias")
            nc.scalar.dma_start(bs[:], bias_t[:])
            fms = wp.tile([128, N_SLOTS * T7u], F32, tag="fmask")
            nc.scalar.dma_start(fms[:], fmask_t[:])

            # ---- concatenated slot-interleaved tail buffers (phase-split) ----
            p4Ecat = cp.tile([96, N_SLOTS * P4E], F32R, tag="p4Ecat")
            p4Ocat = cp.tile([96, N_SLOTS * P4O], F32R, tag="p4Ocat")
            nc.gpsimd.memset(p4Ecat[:].bitcast(F32), 0.0)
            nc.gpsimd.memset(p4Ocat[:].bitcast(F32), 0.0)
            p5Ecat = cp.tile([96, N_SLOTS * P5E], F32R, tag="p5Ecat")
            p5Ocat = cp.tile([96, N_SLOTS * P5O], F32R, tag="p5Ocat")
            b6Ecat = cp.tile([96, N_SLOTS * P6E], F32R, tag="b6Ecat")
            b6Ocat = cp.tile([96, N_SLOTS * P6O], F32R, tag="b6Ocat")
            b7cat = cp.tile([128, N_SLOTS * T7u], F32, tag="b7cat")

            def act(dst_ap, src_ap, bias_col, func=AF.Relu, prange=96):
                nc.scalar.activation(
                    dst_ap, src_ap, func, bias=bs[0:prange, bias_col : bias_col + 1]
                )

            # per-slot phase-split activation buffers
            bufEt = {}
            bufOt = {}

            def emit_l1(j):
                x2 = x2t[j]
                t1 = T1[j]
                bE = ap_.tile([96, t1 // 2], F32R, tag="b1E", name=f"b1E_{j}")
                bO = ap_.tile([96, t1 // 2], F32R, tag="b1O", name=f"b1O_{j}")
                for ts_ in range(0, t1, 2048):
                    cols = min(2048, t1 - ts_)
                    ps = pp.tile([96, 2048], F32, tag="conv", name=f"ps1_{j}_{ts_}")
                    for g in range(5):
                        lhsT = w1s[:, 96 * g : 96 * (g + 1)]
                        for cs in range(0, cols, 512):
                            cn = min(512, cols - cs)
                            t_off = ts_ + cs
                            nc.tensor.matmul(
                                ps[0:96, cs : cs + cn],
                                lhsT,
                                x2[0:80, t_off + g : t_off + g + cn],
                                start=(g == 0),
                                stop=(g == 4),
                            )
                    # phase-split eviction (cols even; ts_ multiple of 2048)
                    h = cols // 2
                    act(bE[:, ts_ // 2 : ts_ // 2 + h], ps[0:96, 0 : cols - 1 : 2], 0)
                    act(bO[:, ts_ // 2 : ts_ // 2 + h], ps[0:96, 1 : cols : 2], 0)
                bufEt[(1, j)] = bE
                bufOt[(1, j)] = bO

            # tap order for K=5 on phase-split input: (E,0),(O,0),(E,1),(O,1),(E,2)
            PHASES5 = ((0, 0), (1, 0), (0, 1), (1, 1), (0, 2))

            def emit_conv(j, lsrc, ldst, w_sb, tout_all, bias_col):
                tout = tout_all[j]
                srcE, srcO = bufEt[(lsrc, j)], bufOt[(lsrc, j)]
                bE = ap_.tile([96, tout // 2], F32R, tag=f"b{ldst}E", name=f"b{ldst}E_{j}")
                bO = ap_.tile([96, tout // 2], F32R, tag=f"b{ldst}O", name=f"b{ldst}O_{j}")
                for ts_ in range(0, tout, 2048):
                    cols = min(2048, tout - ts_)
                    ps = pp.tile([96, 2048], F32, tag="conv", name=f"psc{ldst}_{j}_{ts_}")
                    for g, (ph, d) in enumerate(PHASES5):
                        lhsT = w_sb[:, 96 * g : 96 * (g + 1)]
                        src = srcO if ph else srcE
                        for cs in range(0, cols, 512):
                            cn = min(512, cols - cs)
                            t_off = ts_ + cs
                            nc.tensor.matmul(
                                ps[0:96, cs : cs + cn],
                                lhsT,
                                src[0:96, t_off + d : t_off + d + cn],
                                start=(g == 0),
                                stop=(g == 4),
                            )
                    h = cols // 2
                    act(bE[:, ts_ // 2 : ts_ // 2 + h], ps[0:96, 0 : cols - 1 : 2], bias_col)
                    act(bO[:, ts_ // 2 : ts_ // 2 + h], ps[0:96, 1 : cols : 2], bias_col)
                bufEt[(ldst, j)] = bE
                bufOt[(ldst, j)] = bO

            def emit_l4pool(j):
                t4 = T4[j]
                t4p = T4p[j]
                srcE, srcO = bufEt[(3, j)], bufOt[(3, j)]
                ps = pp.tile([96, 512], F32, tag="conv", name=f"ps4_{j}")
                for g, (ph, d) in enumerate(PHASES5):
                    src = srcO if ph else srcE
                    nc.tensor.matmul(
                        ps[0:96, 0:t4],
                        wls[4][:, 96 * g : 96 * (g + 1)],
                        src[0:96, d : d + t4],
                        start=(g == 0),
                        stop=(g == 4),
                    )
                # pool pairs are exactly (even, odd) psum columns
                tE = ap_.tile([96, t4p], F32, tag="t4e", name=f"t4e_{j}")
                tO = ap_.tile([96, t4p], F32, tag="t4o", name=f"t4o_{j}")
                nc.scalar.activation(tE[:, 0:t4p], ps[0:96, 0 : 2 * t4p - 1 : 2], AF.Copy)
                nc.scalar.activation(tO[:, 0:t4p], ps[0:96, 1 : 2 * t4p : 2], AF.Copy)
                t4s = ap_.tile([96, t4p], F32, tag="t4s", name=f"t4s_{j}")
                nc.vector.tensor_add(t4s[:, 0:t4p], tE[0:96, 0:t4p], tO[0:96, 0:t4p])
                # relu(e+o+2*b4), phase-split into slot-interleaved cat buffers
                nE = (t4p + 1) // 2
                nO = t4p // 2
                act(p4Ecat[0:96, j : 8 * (nE - 1) + j + 1 : 8],
                    t4s[0:96, 0 : 2 * nE - 1 : 2], 3)
                if nO:
                    act(p4Ocat[0:96, j : 8 * (nO - 1) + j + 1 : 8],
                        t4s[0:96, 1 : 2 * nO : 2], 3)

            # ---- paired slot emission ----
            for p in range(0, N_SLOTS, 2):
                a, b = p, p + 1
                emit_l1(a)
                emit_l1(b)
                if p + 2 < N_SLOTS:
                    emit_load(p + 2)
                    emit_load(p + 3)
                emit_conv(a, 1, 2, wls[2], T2, 1)
                emit_conv(b, 1, 2, wls[2], T2, 1)
                emit_conv(a, 2, 3, wls[3], T3, 2)
                emit_conv(b, 2, 3, wls[3], T3, 2)
                emit_l4pool(a)
                emit_l4pool(b)

            # ---- batched tail layers (slot-interleaved, contiguous rhs) ----
            # L5: out col = t*8+s; groups read contiguous slabs of p4{E,O}cat
            ps5 = pp.tile([96, 1024], F32, tag="conv")
            for g, (ph, d) in enumerate(PHASES5):
                lhsT = wls[5][:, 96 * g : 96 * (g + 1)]
                src = p4Ocat if ph else p4Ecat
                for ci, tb in enumerate(range(0, T5u, 64)):
                    tn = min(64, T5u - tb)
                    nc.tensor.matmul(
                        ps5[0:96, 512 * ci : 512 * ci + 8 * tn],
                        lhsT,
                        src[0:96, 8 * (d + tb) : 8 * (d + tb + tn)],
                        start=(g == 0),
                        stop=(g == 4),
                    )
            # pool5: pairs (t=2u, 2u+1) -> cols (16u..16u+7), (16u+8..16u+15)
            buf5 = ap_.tile([96, N_SLOTS * T5u], F32, tag="b5")
            for ci, tb in enumerate(range(0, T5u, 64)):
                tn = min(64, T5u - tb)
                nc.scalar.activation(
                    buf5[:, 8 * tb : 8 * (tb + tn)],
                    ps5[0:96, 512 * ci : 512 * ci + 8 * tn],
                    AF.Copy,
                )
            b5v = buf5[:].rearrange("p (t s) -> p t s", s=N_SLOTS)
            tmp5 = ap_.tile([96, N_SLOTS * T5pu], F32, tag="t5")
            nc.vector.tensor_add(
                tmp5[:].rearrange("p (t s) -> p t s", s=N_SLOTS),
                b5v[:, 0 : 2 * T5pu : 2, :],
                b5v[:, 1 : 2 * T5pu : 2, :],
            )
            # relu + phase-split into p5{E,O}cat (slot-interleaved)
            t5v = tmp5[:].rearrange("p (u s) -> p u s", s=N_SLOTS)
            act(p5Ecat[0:96, :].rearrange("p (u s) -> p u s", s=N_SLOTS),
                t5v[:, 0 : 2 * P5E - 1 : 2, :], 4)
            act(p5Ocat[0:96, :].rearrange("p (u s) -> p u s", s=N_SLOTS),
                t5v[:, 1 : 2 * P5O : 2, :], 4)

            # L6
            ps6 = pp.tile([96, N_SLOTS * T6u], F32, tag="conv")
            for g, (ph, d) in enumerate(PHASES5):
                src = p5Ocat if ph else p5Ecat
                nc.tensor.matmul(
                    ps6[0:96, :],
                    wls[6][:, 96 * g : 96 * (g + 1)],
                    src[0:96, 8 * d : 8 * (d + T6u)],
                    start=(g == 0),
                    stop=(g == 4),
                )
            # relu + phase-split into b6{E,O}cat
            act(b6Ecat[0:96, :].rearrange("p (v s) -> p v s", s=N_SLOTS),
                ps6[0:96, :].rearrange("p (t s) -> p t s", s=N_SLOTS)[:, 0 : 2 * P6E - 1 : 2, :],
                5)
            act(b6Ocat[0:96, :].rearrange("p (v s) -> p v s", s=N_SLOTS),
                ps6[0:96, :].rearrange("p (t s) -> p t s", s=N_SLOTS)[:, 1 : 2 * P6O : 2, :],
                5)

            # L7: taps (E,0),(O,0),(E,1)
            ps7 = pp.tile([128, N_SLOTS * T7u], F32, tag="conv")
            for g, (src, d) in enumerate(((b6Ecat, 0), (b6Ocat, 0), (b6Ecat, 1))):
                nc.tensor.matmul(
                    ps7[0:128, :],
                    w7s[:, 128 * g : 128 * (g + 1)],
                    src[0:96, 8 * d : 8 * (d + T7u)],
                    start=(g == 0),
                    stop=(g == 2),
                )
            act(b7cat[:], ps7[0:128, :], 6, prange=128)  # col = t*8+s

            # ---- ragged masked max + MLP head ----
            tmpm = ap_.tile([128, N_SLOTS * T7u], F32, tag="tm")
            nc.vector.tensor_add(tmpm[:], b7cat[:], fms[:])
            xmax = ap_.tile([128, N_SLOTS], F32R, tag="xmax")
            nc.vector.reduce_max(
                xmax[:],
                tmpm[:].rearrange("p (t s) -> p s t", s=N_SLOTS),
                axis=AX.X,
            )

            psm1 = pp.tile([128, N_SLOTS], F32, tag="conv")
            nc.tensor.matmul(psm1[0:128, :], lw1s[:], xmax[:], start=True, stop=True)
            h1 = ap_.tile([128, N_SLOTS], F32R, tag="h1")
            act(h1[:], psm1[0:128, :], 7, prange=128)

            psm2 = pp.tile([64, N_SLOTS], F32, tag="conv")
            nc.tensor.matmul(psm2[0:64, :], lw2s[:], h1[:], start=True, stop=True)
            h2 = ap_.tile([64, N_SLOTS], F32R, tag="h2")
            act(h2[:], psm2[0:64, :], 8, prange=64)

            psm3 = pp.tile([5, N_SLOTS], F32, tag="conv")
            nc.tensor.matmul(psm3[0:5, :], lw3s[:], h2[0:64, :], start=True, stop=True)
            outsb = ap_.tile([5, N_SLOTS], F32, tag="osb")
            nc.vector.tensor_scalar_add(outsb[:], psm3[0:5, :], bs[0:5, 9:10])
            nc.sync.dma_start(out_t[:], outsb[:])

    nc.compile()
    return nc


def _prep_x(x, b, cap):
    """Host-side input re-layout: phase-major polyphase [80, cap//2]."""
    xb = np.asarray(x[b, :, :cap], np.float32)
    th = cap // 2
    return np.concatenate([xb[:, 0 : 2 * th : 2], xb[:, 1 : 2 * th : 2]], axis=0)


def _prep_weights(inp):
    """Host-side weight/bias re-layout (all tiny)."""
    w = {}
    w1 = np.asarray(inp["w1"], np.float32)  # [96, 40, 10]
    # polyphase phase-major rows (p*40+c), cols (m*96+o): W1[o, c, 2m+p]
    w["w1s"] = np.ascontiguousarray(
        w1.transpose(1, 2, 0).reshape(40, 5, 2, 96).transpose(2, 0, 1, 3).reshape(80, 480)
    )
    for l, scale in ((2, 1.0), (3, 1.0), (4, 1.0), (5, 0.5), (6, 0.5)):
        wl = np.asarray(inp[f"w{l}"], np.float32)  # [96, 96, 5]
        w[f"w{l}s"] = np.ascontiguousarray(wl.transpose(1, 2, 0).reshape(96, 480) * scale)
    w7 = np.asarray(inp["w7"], np.float32)  # [128, 96, 3]
    w["w7s"] = np.ascontiguousarray(w7.transpose(1, 2, 0).reshape(96, 384))
    w["lw1T"] = np.ascontiguousarray(np.asarray(inp["lw1"], np.float32).T)  # [128,128]
    w["lw2T"] = np.ascontiguousarray(np.asarray(inp["lw2"], np.float32).T)  # [128,64]
    w["lw3T"] = np.ascontiguousarray(np.asarray(inp["lw3"], np.float32).T)  # [64,5]

    biases = np.zeros((128, 10), np.float32)
    biases[0:96, 0] = np.asarray(inp["b1"], np.float32)
    biases[0:96, 1] = np.asarray(inp["b2"], np.float32)
    biases[0:96, 2] = np.asarray(inp["b3"], np.float32)
    biases[0:96, 3] = 2.0 * np.asarray(inp["b4"], np.float32)
    biases[0:96, 4] = 2.0 * np.asarray(inp["b5"], np.float32)
    biases[0:96, 5] = np.asarray(inp["b6"], np.float32)
    biases[0:128, 6] = np.asarray(inp["b7"], np.float32)
    biases[0:128, 7] = np.asarray(inp["lb1"], np.float32)
    biases[0:64, 8] = np.asarray(inp["lb2"], np.float32)
    biases[0:5, 9] = np.asarray(inp["lb3"], np.float32)
    w["biases"] = biases
    return w


def _schedule(len_mask):
    """Sort samples by length desc, deal round-robin: core c, slot j gets
    sample order[8j + c].  Slot capacity = rank-group max."""
    lens = np.asarray(len_mask, np.int64).clip(1, T_FULL)
    order = np.argsort(-lens, kind="stable")
    sample_of = np.zeros((N_CORES, N_SLOTS), np.int64)
    caps = []
    for j in range(N_SLOTS):
        grp = order[j * N_CORES : (j + 1) * N_CORES]
        for c in range(N_CORES):
            sample_of[c, j] = grp[c]
        cap = int(lens[grp].max())
        cap = max(cap, 1312)  # keep the whole chain >= 1 frame
        # round up to a multiple of 32 so T1..T4 are all even
        # (fp32r matmuls require an even moving-operand size)
        cap = min(((cap + 31) // 32) * 32, T_FULL)
        caps.append(cap)
    return order, sample_of, caps


def _make_in_maps(inputs, sample_of, caps):
    x = np.asarray(inputs["x_input"], np.float32)
    len_mask = np.asarray(inputs["len_mask"], np.int32)
    _, _, _, _, T7u = _uniform_tail(caps)
    w = _prep_weights(inputs)
    in_maps = []
    for c in range(N_CORES):
        m = dict(w)
        # slot-interleaved mask layout: column = t*8 + s
        fm2 = np.full((T7u, N_SLOTS), NEG, np.float32)
        for j in range(N_SLOTS):
            bidx = int(sample_of[c, j])
            m[f"x{j}"] = _prep_x(x, bidx, caps[j])
            lv7 = _chain(int(max(min(len_mask[bidx], T_FULL), 1312)))[8]
            lv7 = max(min(lv7, T7u), 1)
            fm2[0:lv7, j] = 0.0
        fmask = fm2.reshape(-1)
        m["fmask"] = np.ascontiguousarray(
            np.broadcast_to(fmask[None, :], (128, N_SLOTS * T7u))
        )
        in_maps.append(m)
    return in_maps


def _ensure_ntff_hook():
    """The agent image lacks ``antenv.axon_hooks``; seed a shim so
    ``run_bass_kernel_spmd(trace=True)`` can reach the axon NTFF profiler."""
    import types

    if "antenv.axon_hooks" in sys.modules:
        return
    try:
        from trn_agent_boot.trn_boot import _ntff_profile_via_ctypes

        hook = _ntff_profile_via_ctypes("/opt/axon/libaxon_pjrt.so")
    except Exception:
        hook = None
    mod = types.ModuleType("antenv.axon_hooks")
    state = {"hook": hook}
    mod.get_axon_ntff_profile_hook = lambda: state["hook"]
    mod.set_axon_ntff_profile_hook = lambda h: state.update(hook=h)
    sys.modules["antenv.axon_hooks"] = mod


_LDW_PATCHED = False


def _enable_ldw_opt():
    """Turn on walrus's LDWEIGHTS dedup (drops redundant weight reloads for
    back-to-back same-weight matmuls).  Verified bit-identical results."""
    global _LDW_PATCHED
    if _LDW_PATCHED:
        return
    try:
        import concourse.bass_utils as bu

        _orig = bu.run_command

        def run_command_ldw(argv, **kw):
            argv = [
                "--enable-ldw-opt=true" if a == "--enable-ldw-opt=false" else a
                for a in argv
            ]
            return _orig(argv, **kw)

        bu.run_command = run_command_ldw
        _LDW_PATCHED = True
    except Exception:
        pass


def _run(inputs, trace=False):
    if trace:
        _ensure_ntff_hook()
    _enable_ldw_opt()
    len_mask = np.asarray(inputs["len_mask"], np.int32)
    order, sample_of, caps = _schedule(len_mask)
    nc = _build_program(caps)
    in_maps = _make_in_maps(inputs, sample_of, caps)
    res = run_bass_kernel_spmd(
        nc, in_maps, core_ids=list(range(N_CORES)), trace=trace
    )
    out = np.zeros((B, 5), np.float32)
    for c in range(N_CORES):
        o = res.results[c]["out"]  # [5, 8]
        for j in range(N_SLOTS):
            out[int(sample_of[c, j])] = o[:, j]
    return out, res


def kernel(**inputs):
    out, _ = _run(inputs, trace=False)
    return out
